# revision 1
# baseline (speedup 1.0000x reference)
"""Bilinear interaction layer (nn_BilinearInteractionLayer) on 8 TRN2 cores.

out[b, p*64+e] = (sum_d x[b, i_p, d] * W[p, d, e]) * x[b, j_p, e]
  with (i_p, j_p) the p-th pair of triu_indices(32, k=1), B=2048, D=64, P=496.

Sharding: data-parallel over batch (8 x 256 rows); W replicated on every core.
kernel(**inputs) takes the FULL inputs, shards on host, runs one SPMD Bass
program on cores 0..7 via run_bass_kernel_spmd, and concatenates the per-core
[256, 31744] outputs back to [2048, 31744] (float32, matching the reference).

Per-core kernel. Matmul form out[b,e] = xT_i.T @ W[p] puts the result in
natural [batch, e] layout, so the vj elementwise multiply and the output DMA
need no further transposes and every output DMA row is a contiguous DRAM run:
  - x natural [256, 2048] in SBUF (the vj operand of the multiply)
  - xt host-pretransposed [128, 4096]: rows 0:64 hold even features as
    [d, batch], rows 64:128 odd features. Stationary (lhsT) tiles [64, 128].
  - W host-packed [128, 16384]: rows 0:64 = the 256 even-i pairs' [d, e]
    blocks, rows 64:128 = the 240 odd-i pairs (zero-padded). The moving (rhs)
    operand for one matmul is 8 consecutive pairs = [64, 512].
  - K=64 matmuls run on PE row halves 0:64 / 64:128 (tile_position derives
    from the operand base partition), so even-i and odd-i matmuls overlap on
    the array.
  - Matmul outputs land packed in multi-bank PSUM tiles; the DVE multiplies
    each PSUM block by the matching contiguous slice of x (j runs
    consecutively within an i-block) straight into an SBUF staging tile;
    one output DMA per (b_tile, adjacent-i-block-pair) writes [128 rows x
    up to 15.6KB] contiguous chunks.
"""

import numpy as np

F = 32
D = 64
NPAIR = F * (F - 1) // 2  # 496
B = 2048
NCORES = 8
BS = B // NCORES  # 256
PD = NPAIR * D  # 31744

_EVEN_I = list(range(0, F - 1, 2))  # 0..30
_ODD_I = list(range(1, F - 1, 2))  # 1..29 (31 has no pairs)


def _off(i):
    # start pair-index of the i-block in natural triu order
    return (F - 1) * i - i * (i - 1) // 2


def _cum(idx_list):
    c, out = 0, {}
    for i in idx_list:
        out[i] = c
        c += (F - 1) - i
    return out, c


_CUM_EVEN, _N_EVEN = _cum(_EVEN_I)  # 256
_CUM_ODD, _N_ODD = _cum(_ODD_I)  # 240

_NC_CACHE = {}

# Kernel variant. Base dtype: "float32" (bit-exact fp32, PE streams 4 cyc/col)
# or "f32r" (FP32R single-pass, 1 cyc/col, tf32-class rounding, ~2.2e-4
# scale-relative absmax err vs fp32 reference). Suffixes: "_bigdve2" batches
# matmul outputs into 2-bank PSUM tiles so the vj elementwise multiply runs as
# ~76 large DVE ops instead of 140 (DVE is the #2 engine); "_notr" transposes
# x on the PE instead of shipping a host-pretransposed copy.
# "_v4" additionally orders input DMAs in first-consumption order (x, then xt
# and W in round-sized chunks) so the first matmul issues ~10us in instead of
# ~35us, uses 4 staging bufs, and trims the odd-half W zero padding.
# Measured (8 cores, per-iteration HW time, same-session comparisons; absolute
# numbers vary ~66-120us with host load):
#   float32 114-297us | f32r 91-123 | bigdve2 91.5-119.6 | v4 ~ -5 | v6 best
# "_v6" merges the small late output rounds (k>=8) in pairs: 12 output DMAs
# per b_tile instead of 16, tail chunks 2x bigger, same max staging tile.
DTYPE = "f32r_v6"


def _build_nc(dtype_name="float32", repeat=1):
    import concourse.mybir as mybir
    import concourse.tile as tile
    from concourse import bacc

    key = (dtype_name, repeat)
    if key in _NC_CACHE:
        return _NC_CACHE[key]

    f32 = mybir.dt.float32
    # float32r: PE streams 1 col/cycle (vs 4 for plain fp32) at tf32-class
    # precision (~1.6e-4 rel err measured); float32 is bit-exact vs reference.
    base, _, suffix = dtype_name.partition("_")
    mm_dt = mybir.dt.float32r if base == "f32r" else f32
    v7 = "v7" in suffix  # v6 + quad-merge the tail rounds + 5 staging bufs
    v6 = v7 or "v6" in suffix  # v4 + merge only the small late rounds (k>=8)
    v5 = "v5" in suffix  # v4 + merge 2 k-rounds per staging tile / out-DMA
    v4 = v5 or v6 or "v4" in suffix  # v3 + chunked-xt + early-W DMA order
    v3 = v4 or "v3" in suffix  # bigdve2 + x/xt-before-W DMA order + 4 stg bufs
    if v3:
        suffix = suffix + "_bigdve2"
    on_chip_tr = "notr" in suffix  # transpose x on the PE instead of host xt
    big_dve = "bigdve" in suffix  # multi-bank PSUM tiles + fewer, larger DVE ops
    ps_banks = 2 if ("bigdve2" in suffix or on_chip_tr) else 4
    ps_bufs = (8 // ps_banks) if big_dve else (5 if on_chip_tr else 6)
    if big_dve and on_chip_tr:
        ps_bufs = 3  # 3*2 banks + 2 transpose banks = 8
    op_bufs = 3 if v5 else (5 if v7 else (4 if v3 else 3))
    if v7:
        k_groups = (
            [(k, k + 1) for k in range(4)]
            + [(k, k + 2) for k in range(4, 12, 2)]
            + [(12, 16)]
        )
    elif v5:
        k_groups = [(k, k + 2) for k in range(0, 16, 2)]
    elif v6:
        k_groups = [(k, k + 1) for k in range(8)] + [(k, k + 2) for k in range(8, 16, 2)]
    else:
        k_groups = [(k, k + 1) for k in range(16)]
    nc = bacc.Bacc("TRN2", target_bir_lowering=False, debug=False)

    x_d = nc.dram_tensor("x", [BS, F * D], f32, kind="ExternalInput")
    xt_d = ident_d = None
    if on_chip_tr:
        ident_d = nc.dram_tensor("ident", [128, 128], f32, kind="ExternalInput")
    else:
        xt_d = nc.dram_tensor("xt", [128, 16 * BS], f32, kind="ExternalInput")
    w_d = nc.dram_tensor("w", [128, _N_EVEN * D], f32, kind="ExternalInput")
    y_d = nc.dram_tensor("y", [BS, PD], f32, kind="ExternalOutput")

    with tile.TileContext(nc) as tc:
        import contextlib

        with (
            tc.tile_pool(name="const", bufs=1) as const,
            tc.tile_pool(name="xp", bufs=2) as xpool,
            tc.tile_pool(name="ps", bufs=ps_bufs, space="PSUM") as pspool,
            tc.tile_pool(name="ps2", bufs=2, space="PSUM") as pspool2,
            tc.tile_pool(name="op", bufs=op_bufs) as opool,
            (tc.For_i(0, repeat, 1) if repeat > 1 else contextlib.nullcontext()),
        ):
            w_buf = const.tile([128, _N_EVEN * D], mm_dt, tag="w")
            xt_buf = const.tile([128, 16 * BS], mm_dt, tag="xt")
            ident = None
            x_tiles = {}
            wcols = _N_EVEN * D
            if v4:
                # Finest-grained first-consumption ordering: round k needs xt
                # cols [k*256,(k+1)*256) and W pair-cols up to cum(2k)+...;
                # stream both in chunks interleaved so the first matmul starts
                # ~6us in, and trim the odd-half zero padding off the last W
                # chunk (only 240*64 of 256*64 cols are real).
                for t in range(BS // 128):
                    x_tiles[t] = xpool.tile(
                        [128, F * D], mm_dt, tag="x", name=f"x{t}"
                    )
                nc.sync.dma_start(x_tiles[0][:, :], x_d[0:128, :].bitcast(mm_dt))
                xtc = 16 * BS // 4  # 1024 cols = rounds 4k..4k+3
                nc.sync.dma_start(
                    xt_buf[:, 0:xtc], xt_d[:, 0:xtc].bitcast(mm_dt)
                )
                wc = wcols // 8  # 2048 cols = 32 pairs per half
                nc.sync.dma_start(w_buf[:, 0:wc], w_d[:, 0:wc].bitcast(mm_dt))
                nc.sync.dma_start(x_tiles[1][:, :], x_d[128:256, :].bitcast(mm_dt))
                nc.sync.dma_start(
                    xt_buf[:, xtc : 2 * xtc], xt_d[:, xtc : 2 * xtc].bitcast(mm_dt)
                )
                nc.sync.dma_start(
                    w_buf[:, wc : 2 * wc], w_d[:, wc : 2 * wc].bitcast(mm_dt)
                )
                nc.sync.dma_start(
                    xt_buf[:, 2 * xtc :], xt_d[:, 2 * xtc :].bitcast(mm_dt)
                )
                for q in range(2, 8):
                    c0, c1 = q * wc, (q + 1) * wc
                    if q < 7:
                        nc.sync.dma_start(
                            w_buf[:, c0:c1], w_d[:, c0:c1].bitcast(mm_dt)
                        )
                    else:
                        # last chunk: odd half (rows 64:128) is zero-padded
                        # past col _N_ODD*D — skip the padding bytes.
                        nc.sync.dma_start(
                            w_buf[0:64, c0:c1], w_d[0:64, c0:c1].bitcast(mm_dt)
                        )
                        nc.sync.dma_start(
                            w_buf[64:128, c0 : _N_ODD * D],
                            w_d[64:128, c0 : _N_ODD * D].bitcast(mm_dt),
                        )
            elif v3:
                # Issue input DMAs in first-consumption order: x_t0 and xt
                # unblock the first matmul+multiply ~20us earlier than loading
                # all of W first; W streams in 1MB chunks behind them.
                for t in range(BS // 128):
                    x_tiles[t] = xpool.tile(
                        [128, F * D], mm_dt, tag="x", name=f"x{t}"
                    )
                nc.sync.dma_start(
                    x_tiles[0][:, :], x_d[0:128, :].bitcast(mm_dt)
                )
                nc.sync.dma_start(xt_buf[:, :], xt_d[:, :].bitcast(mm_dt))
                nc.sync.dma_start(
                    w_buf[:, 0 : wcols // 8], w_d[:, 0 : wcols // 8].bitcast(mm_dt)
                )
                nc.sync.dma_start(
                    x_tiles[1][:, :], x_d[128:256, :].bitcast(mm_dt)
                )
                for q in range(1, 8):
                    c0, c1 = q * wcols // 8, (q + 1) * wcols // 8
                    nc.sync.dma_start(w_buf[:, c0:c1], w_d[:, c0:c1].bitcast(mm_dt))
            else:
                if on_chip_tr:
                    # DMA the identity (host np.eye) rather than memset+affine
                    # -select: those ops reject f32r in walrus codegen.
                    ident = const.tile([128, 128], mm_dt, tag="ident")
                    nc.sync.dma_start(ident[:, :], ident_d[:, :].bitcast(mm_dt))
                else:
                    nc.sync.dma_start(xt_buf[:, :], xt_d[:, :].bitcast(mm_dt))
                for q in range(4):
                    c0, c1 = q * wcols // 4, (q + 1) * wcols // 4
                    nc.sync.dma_start(w_buf[:, c0:c1], w_d[:, c0:c1].bitcast(mm_dt))

            for t in range(BS // 128):
                if v3:
                    x_tile = x_tiles[t]
                else:
                    x_tile = xpool.tile([128, F * D], mm_dt, tag="x")
                    nc.sync.dma_start(
                        x_tile[:, :], x_d[t * 128 : (t + 1) * 128, :].bitcast(mm_dt)
                    )

                if on_chip_tr:
                    # x_tile cols f*128..(f+1)*128 cover features (2f, 2f+1);
                    # PE transpose -> PSUM [128 d-pair, 128 b]: partitions 0:64
                    # = feature 2f, 64:128 = feature 2f+1 — exactly xt layout.
                    for f in range(16):
                        tp = pspool2.tile([128, 128], mm_dt, tag="tp")
                        nc.tensor.transpose(
                            tp[:, :],
                            x_tile[:, f * 128 : (f + 1) * 128],
                            ident[:, :],
                        )
                        nc.vector.tensor_copy(
                            xt_buf[:, f * BS + t * 128 : f * BS + t * 128 + 128],
                            tp[:, :],
                        )

                for k0, k_end in k_groups:
                  total_m = _off(2 * k_end) - _off(2 * k0)
                  stg = opool.tile([128, total_m * D], f32, tag="stg")
                  for k in range(k0, k_end):
                    ilo, ihi = 2 * k, 2 * k + 1
                    sbase = (_off(ilo) - _off(2 * k0)) * D
                    np_lo = (F - 1) - ilo
                    np_hi = (F - 1) - ihi if ihi < F - 1 else 0
                    total = np_lo + np_hi

                    glo = [(s, min(8, np_lo - s)) for s in range(0, np_lo, 8)]
                    ghi = [(s, min(8, np_hi - s)) for s in range(0, np_hi, 8)]

                    if big_dve:
                        # One PSUM tile (up to ps_banks banks) per half-round;
                        # each group MM targets a bank-aligned slice; one DVE
                        # multiply per psum tile (chunks of ps_banks*8 pairs).
                        halves = [("lo", ilo, sbase, 0, np_lo, glo)]
                        if np_hi:
                            halves.append(
                                ("hi", ihi, sbase + np_lo * D, 64, np_hi, ghi)
                            )
                        chunk_pairs = ps_banks * 8
                        ps_tiles = {}  # (half, chunk_idx) -> tile
                        dve_jobs = []
                        for half, i, base, r0, npair, groups in halves:
                            for c0p in range(0, npair, chunk_pairs):
                                cp = min(chunk_pairs, npair - c0p)
                                pst = pspool.tile(
                                    [128, ps_banks * 512], f32, tag="ps", name="psbig"
                                )
                                ps_tiles[(half, c0p // chunk_pairs)] = pst
                                dve_jobs.append((half, i, base, c0p, cp, pst))
                        # interleave lo/hi MMs for PE row-half overlap
                        seq = []
                        for idx in range(max(len(glo), len(ghi))):
                            for half_info in halves:
                                if idx < len(half_info[5]):
                                    seq.append((half_info, half_info[5][idx]))
                        for (half, i, base, r0, npair, groups), (s, gs) in seq:
                            n = gs * D
                            gidx = (_CUM_EVEN[i] if half == "lo" else _CUM_ODD[i]) + s
                            fi = i // 2
                            lhsT = xt_buf[
                                r0 : r0 + 64,
                                fi * BS + t * 128 : fi * BS + t * 128 + 128,
                            ]
                            rhs = w_buf[r0 : r0 + 64, gidx * D : gidx * D + n]
                            pst = ps_tiles[(half, s // chunk_pairs)]
                            so = (s % chunk_pairs) * D
                            nc.tensor.matmul(
                                pst[:, so : so + n],
                                lhsT,
                                rhs,
                                start=True,
                                stop=True,
                            )
                        for half, i, base, c0p, cp, pst in dve_jobs:
                            nc.vector.tensor_mul(
                                out=stg[:, base + c0p * D : base + (c0p + cp) * D],
                                in0=pst[:, : cp * D],
                                in1=x_tile[
                                    :, (i + 1 + c0p) * D : (i + 1 + c0p + cp) * D
                                ].bitcast(f32),
                            )
                    else:
                        seq = []
                        for idx in range(max(len(glo), len(ghi))):
                            if idx < len(glo):
                                seq.append(("lo", glo[idx]))
                            if idx < len(ghi):
                                seq.append(("hi", ghi[idx]))

                        for half, (s, gs) in seq:
                            n = gs * D
                            if half == "lo":
                                i, base, r0 = ilo, sbase, 0
                                gidx = _CUM_EVEN[i] + s
                            else:
                                i, base, r0 = ihi, sbase + np_lo * D, 64
                                gidx = _CUM_ODD[i] + s
                            fi = i // 2
                            j0 = i + 1 + s
                            ps = pspool.tile([128, 512], f32, tag="ps")
                            lhsT = xt_buf[
                                r0 : r0 + 64,
                                fi * BS + t * 128 : fi * BS + t * 128 + 128,
                            ]
                            rhs = w_buf[r0 : r0 + 64, gidx * D : gidx * D + n]
                            nc.tensor.matmul(
                                ps[:, :n], lhsT, rhs, start=True, stop=True
                            )
                            nc.vector.tensor_mul(
                                out=stg[:, base + s * D : base + s * D + n],
                                in0=ps[:, :n],
                                in1=x_tile[:, j0 * D : j0 * D + n].bitcast(f32),
                            )

                    if k == k_end - 1:
                        c0 = _off(2 * k0) * D
                        nc.sync.dma_start(
                            y_d[t * 128 : (t + 1) * 128, c0 : c0 + total_m * D],
                            stg[:, :],
                        )

    nc.finalize()
    _NC_CACHE[key] = nc
    return nc


def _prep_inputs(inputs, W, host_xt=True):
    inputs = np.ascontiguousarray(np.asarray(inputs, dtype=np.float32))
    W = np.ascontiguousarray(np.asarray(W, dtype=np.float32))

    even_p = [p for p, i in enumerate(_pair_i()) if i % 2 == 0]
    odd_p = [p for p, i in enumerate(_pair_i()) if i % 2 == 1]
    w_packed = np.zeros((128, _N_EVEN * D), dtype=np.float32)
    w_packed[0:64, :] = W[even_p].transpose(1, 0, 2).reshape(64, _N_EVEN * D)
    w_packed[64:128, : _N_ODD * D] = (
        W[odd_p].transpose(1, 0, 2).reshape(64, _N_ODD * D)
    )

    in_maps = []
    for c in range(NCORES):
        xs = inputs[c * BS : (c + 1) * BS]  # [256, 32, 64]
        x_flat = np.ascontiguousarray(xs.reshape(BS, F * D))
        m = {"x": x_flat, "w": w_packed}
        if not host_xt:
            m["ident"] = np.eye(128, dtype=np.float32)
        if host_xt:
            xtt = xs.transpose(2, 1, 0)  # [64, 32, 256]
            xt = np.empty((128, 16 * BS), dtype=np.float32)
            xt[0:64, :] = np.ascontiguousarray(xtt[:, 0::2, :]).reshape(64, 16 * BS)
            xt[64:128, :] = np.ascontiguousarray(xtt[:, 1::2, :]).reshape(64, 16 * BS)
            m["xt"] = xt
        in_maps.append(m)
    return in_maps


_PAIR_I = None


def _pair_i():
    global _PAIR_I
    if _PAIR_I is None:
        _PAIR_I = [i for i in range(F) for _ in range(i + 1, F)]
    return _PAIR_I


def _run(inputs, W, trace=False, trace_cores=None, dtype_name=None):
    from concourse.bass_utils import run_bass_kernel_spmd

    dn = dtype_name or DTYPE
    nc = _build_nc(dn)
    in_maps = _prep_inputs(inputs, W, host_xt="_notr" not in dn)
    res = run_bass_kernel_spmd(
        nc,
        in_maps,
        core_ids=list(range(NCORES)),
        trace=trace,
        trace_cores=trace_cores,
    )
    out = np.concatenate([res.results[c]["y"] for c in range(NCORES)], axis=0)
    return out, res


def kernel(inputs, W):
    out, _ = _run(inputs, W, trace=False)
    return out



# revision 20
# speedup vs baseline: 1.9230x; 1.9230x over previous
"""Bilinear interaction layer (nn_BilinearInteractionLayer) on 8 TRN2 cores.

out[b, p*64+e] = (sum_d x[b, i_p, d] * W[p, d, e]) * x[b, j_p, e]
  with (i_p, j_p) the p-th pair of triu_indices(32, k=1), B=2048, D=64, P=496.

Sharding: data-parallel over batch (8 x 256 rows); W replicated on every core.
kernel(**inputs) takes the FULL inputs, shards on host, runs one SPMD Bass
program on cores 0..7 via run_bass_kernel_spmd, and concatenates the per-core
[256, 31744] outputs back to [2048, 31744] (float32, matching the reference).

Per-core kernel. Matmul form out[b,e] = xT_i.T @ W[p] puts the result in
natural [batch, e] layout, so the vj elementwise multiply and the output DMA
need no further transposes and every output DMA row is a contiguous DRAM run:
  - x natural [256, 2048] in SBUF (the vj operand of the multiply)
  - xt host-pretransposed [128, 4096]: rows 0:64 hold even features as
    [d, batch], rows 64:128 odd features. Stationary (lhsT) tiles [64, 128].
  - W host-packed [128, 16384]: rows 0:64 = the 256 even-i pairs' [d, e]
    blocks, rows 64:128 = the 240 odd-i pairs (zero-padded). The moving (rhs)
    operand for one matmul is 8 consecutive pairs = [64, 512].
  - K=64 matmuls run on PE row halves 0:64 / 64:128 (tile_position derives
    from the operand base partition), so even-i and odd-i matmuls overlap on
    the array.
  - Matmul outputs land packed in multi-bank PSUM tiles; the DVE multiplies
    each PSUM block by the matching contiguous slice of x (j runs
    consecutively within an i-block) straight into an SBUF staging tile;
    one output DMA per (b_tile, adjacent-i-block-pair) writes [128 rows x
    up to 15.6KB] contiguous chunks.
"""

import numpy as np

F = 32
D = 64
NPAIR = F * (F - 1) // 2  # 496
B = 2048
NCORES = 8
BS = B // NCORES  # 256
PD = NPAIR * D  # 31744

_EVEN_I = list(range(0, F - 1, 2))  # 0..30
_ODD_I = list(range(1, F - 1, 2))  # 1..29 (31 has no pairs)


def _off(i):
    # start pair-index of the i-block in natural triu order
    return (F - 1) * i - i * (i - 1) // 2


def _cum(idx_list):
    c, out = 0, {}
    for i in idx_list:
        out[i] = c
        c += (F - 1) - i
    return out, c


_CUM_EVEN, _N_EVEN = _cum(_EVEN_I)  # 256
_CUM_ODD, _N_ODD = _cum(_ODD_I)  # 240

_NC_CACHE = {}

# Kernel variant. Base dtype: "float32" (bit-exact fp32, PE streams 4 cyc/col)
# or "f32r" (FP32R single-pass, 1 cyc/col, tf32-class rounding, ~2.2e-4
# scale-relative absmax err vs fp32 reference). Suffixes: "_bigdve2" batches
# matmul outputs into 2-bank PSUM tiles so the vj elementwise multiply runs as
# ~76 large DVE ops instead of 140 (DVE is the #2 engine); "_notr" transposes
# x on the PE instead of shipping a host-pretransposed copy.
# "_v4" additionally orders input DMAs in first-consumption order (x, then xt
# and W in round-sized chunks) so the first matmul issues ~10us in instead of
# ~35us, uses 4 staging bufs, and trims the odd-half W zero padding.
# Measured (8 cores, per-iteration HW time, same-session comparisons; absolute
# numbers vary ~66-120us with host load):
#   float32 114-297us | f32r 91-123 | bigdve2 91.5-119.6 | v4 ~ -5 | v6 best
# "_v6" merges the small late output rounds (k>=8) in pairs: 12 output DMAs
# per b_tile instead of 16, tail chunks 2x bigger, same max staging tile.
#
# "fp16_v8": all HBM I/O in fp16 (x, xt, W reads AND the y write; host converts
# y back to fp32). Halves DMA traffic 44.5 -> 22.25 MB/core (~62us roofline at
# 358 GB/s). The elementwise vj multiply is restructured because DVE
# tensor_tensor from PSUM runs at 1x (PSUM has one DVE read port): matmuls
# pack PSUM tiles densely in triu pair order; the SCALAR engine (1.2 GHz, can
# read PSUM) evicts whole 2048-col PSUM tiles into the fp16 staging tile; DVE
# then multiplies staging in place by vj at fp16 2x rate (one op per i-block).
# The 11 small late blocks (i>=20, 13% of cols) are instead multiplied by DVE
# straight from PSUM to keep the scalar engine under the DMA roofline.
# Output: 2 large DMAs per b_tile (4.3MB + 3.8MB).
DTYPE = "fp16_v9"


def _build_nc(dtype_name="float32", repeat=1):
    import concourse.mybir as mybir
    import concourse.tile as tile
    from concourse import bacc

    if dtype_name.startswith("fp16"):
        if "v9" in dtype_name:
            return _build_nc_v9(dtype_name, repeat)
        return _build_nc_v8(dtype_name, repeat)

    key = (dtype_name, repeat)
    if key in _NC_CACHE:
        return _NC_CACHE[key]

    f32 = mybir.dt.float32
    # float32r: PE streams 1 col/cycle (vs 4 for plain fp32) at tf32-class
    # precision (~1.6e-4 rel err measured); float32 is bit-exact vs reference.
    base, _, suffix = dtype_name.partition("_")
    mm_dt = mybir.dt.float32r if base == "f32r" else f32
    v7 = "v7" in suffix  # v6 + quad-merge the tail rounds + 5 staging bufs
    v6 = v7 or "v6" in suffix  # v4 + merge only the small late rounds (k>=8)
    v5 = "v5" in suffix  # v4 + merge 2 k-rounds per staging tile / out-DMA
    v4 = v5 or v6 or "v4" in suffix  # v3 + chunked-xt + early-W DMA order
    v3 = v4 or "v3" in suffix  # bigdve2 + x/xt-before-W DMA order + 4 stg bufs
    if v3:
        suffix = suffix + "_bigdve2"
    on_chip_tr = "notr" in suffix  # transpose x on the PE instead of host xt
    big_dve = "bigdve" in suffix  # multi-bank PSUM tiles + fewer, larger DVE ops
    ps_banks = 2 if ("bigdve2" in suffix or on_chip_tr) else 4
    ps_bufs = (8 // ps_banks) if big_dve else (5 if on_chip_tr else 6)
    if big_dve and on_chip_tr:
        ps_bufs = 3  # 3*2 banks + 2 transpose banks = 8
    op_bufs = 3 if v5 else (5 if v7 else (4 if v3 else 3))
    if v7:
        k_groups = (
            [(k, k + 1) for k in range(4)]
            + [(k, k + 2) for k in range(4, 12, 2)]
            + [(12, 16)]
        )
    elif v5:
        k_groups = [(k, k + 2) for k in range(0, 16, 2)]
    elif v6:
        k_groups = [(k, k + 1) for k in range(8)] + [(k, k + 2) for k in range(8, 16, 2)]
    else:
        k_groups = [(k, k + 1) for k in range(16)]
    nc = bacc.Bacc("TRN2", target_bir_lowering=False, debug=False)

    x_d = nc.dram_tensor("x", [BS, F * D], f32, kind="ExternalInput")
    xt_d = ident_d = None
    if on_chip_tr:
        ident_d = nc.dram_tensor("ident", [128, 128], f32, kind="ExternalInput")
    else:
        xt_d = nc.dram_tensor("xt", [128, 16 * BS], f32, kind="ExternalInput")
    w_d = nc.dram_tensor("w", [128, _N_EVEN * D], f32, kind="ExternalInput")
    y_d = nc.dram_tensor("y", [BS, PD], f32, kind="ExternalOutput")

    with tile.TileContext(nc) as tc:
        import contextlib

        with (
            tc.tile_pool(name="const", bufs=1) as const,
            tc.tile_pool(name="xp", bufs=2) as xpool,
            tc.tile_pool(name="ps", bufs=ps_bufs, space="PSUM") as pspool,
            tc.tile_pool(name="ps2", bufs=2, space="PSUM") as pspool2,
            tc.tile_pool(name="op", bufs=op_bufs) as opool,
            (tc.For_i(0, repeat, 1) if repeat > 1 else contextlib.nullcontext()),
        ):
            w_buf = const.tile([128, _N_EVEN * D], mm_dt, tag="w")
            xt_buf = const.tile([128, 16 * BS], mm_dt, tag="xt")
            ident = None
            x_tiles = {}
            wcols = _N_EVEN * D
            if v4:
                # Finest-grained first-consumption ordering: round k needs xt
                # cols [k*256,(k+1)*256) and W pair-cols up to cum(2k)+...;
                # stream both in chunks interleaved so the first matmul starts
                # ~6us in, and trim the odd-half zero padding off the last W
                # chunk (only 240*64 of 256*64 cols are real).
                for t in range(BS // 128):
                    x_tiles[t] = xpool.tile(
                        [128, F * D], mm_dt, tag="x", name=f"x{t}"
                    )
                nc.sync.dma_start(x_tiles[0][:, :], x_d[0:128, :].bitcast(mm_dt))
                xtc = 16 * BS // 4  # 1024 cols = rounds 4k..4k+3
                nc.sync.dma_start(
                    xt_buf[:, 0:xtc], xt_d[:, 0:xtc].bitcast(mm_dt)
                )
                wc = wcols // 8  # 2048 cols = 32 pairs per half
                nc.sync.dma_start(w_buf[:, 0:wc], w_d[:, 0:wc].bitcast(mm_dt))
                nc.sync.dma_start(x_tiles[1][:, :], x_d[128:256, :].bitcast(mm_dt))
                nc.sync.dma_start(
                    xt_buf[:, xtc : 2 * xtc], xt_d[:, xtc : 2 * xtc].bitcast(mm_dt)
                )
                nc.sync.dma_start(
                    w_buf[:, wc : 2 * wc], w_d[:, wc : 2 * wc].bitcast(mm_dt)
                )
                nc.sync.dma_start(
                    xt_buf[:, 2 * xtc :], xt_d[:, 2 * xtc :].bitcast(mm_dt)
                )
                for q in range(2, 8):
                    c0, c1 = q * wc, (q + 1) * wc
                    if q < 7:
                        nc.sync.dma_start(
                            w_buf[:, c0:c1], w_d[:, c0:c1].bitcast(mm_dt)
                        )
                    else:
                        # last chunk: odd half (rows 64:128) is zero-padded
                        # past col _N_ODD*D — skip the padding bytes.
                        nc.sync.dma_start(
                            w_buf[0:64, c0:c1], w_d[0:64, c0:c1].bitcast(mm_dt)
                        )
                        nc.sync.dma_start(
                            w_buf[64:128, c0 : _N_ODD * D],
                            w_d[64:128, c0 : _N_ODD * D].bitcast(mm_dt),
                        )
            elif v3:
                # Issue input DMAs in first-consumption order: x_t0 and xt
                # unblock the first matmul+multiply ~20us earlier than loading
                # all of W first; W streams in 1MB chunks behind them.
                for t in range(BS // 128):
                    x_tiles[t] = xpool.tile(
                        [128, F * D], mm_dt, tag="x", name=f"x{t}"
                    )
                nc.sync.dma_start(
                    x_tiles[0][:, :], x_d[0:128, :].bitcast(mm_dt)
                )
                nc.sync.dma_start(xt_buf[:, :], xt_d[:, :].bitcast(mm_dt))
                nc.sync.dma_start(
                    w_buf[:, 0 : wcols // 8], w_d[:, 0 : wcols // 8].bitcast(mm_dt)
                )
                nc.sync.dma_start(
                    x_tiles[1][:, :], x_d[128:256, :].bitcast(mm_dt)
                )
                for q in range(1, 8):
                    c0, c1 = q * wcols // 8, (q + 1) * wcols // 8
                    nc.sync.dma_start(w_buf[:, c0:c1], w_d[:, c0:c1].bitcast(mm_dt))
            else:
                if on_chip_tr:
                    # DMA the identity (host np.eye) rather than memset+affine
                    # -select: those ops reject f32r in walrus codegen.
                    ident = const.tile([128, 128], mm_dt, tag="ident")
                    nc.sync.dma_start(ident[:, :], ident_d[:, :].bitcast(mm_dt))
                else:
                    nc.sync.dma_start(xt_buf[:, :], xt_d[:, :].bitcast(mm_dt))
                for q in range(4):
                    c0, c1 = q * wcols // 4, (q + 1) * wcols // 4
                    nc.sync.dma_start(w_buf[:, c0:c1], w_d[:, c0:c1].bitcast(mm_dt))

            for t in range(BS // 128):
                if v3:
                    x_tile = x_tiles[t]
                else:
                    x_tile = xpool.tile([128, F * D], mm_dt, tag="x")
                    nc.sync.dma_start(
                        x_tile[:, :], x_d[t * 128 : (t + 1) * 128, :].bitcast(mm_dt)
                    )

                if on_chip_tr:
                    # x_tile cols f*128..(f+1)*128 cover features (2f, 2f+1);
                    # PE transpose -> PSUM [128 d-pair, 128 b]: partitions 0:64
                    # = feature 2f, 64:128 = feature 2f+1 — exactly xt layout.
                    for f in range(16):
                        tp = pspool2.tile([128, 128], mm_dt, tag="tp")
                        nc.tensor.transpose(
                            tp[:, :],
                            x_tile[:, f * 128 : (f + 1) * 128],
                            ident[:, :],
                        )
                        nc.vector.tensor_copy(
                            xt_buf[:, f * BS + t * 128 : f * BS + t * 128 + 128],
                            tp[:, :],
                        )

                for k0, k_end in k_groups:
                  total_m = _off(2 * k_end) - _off(2 * k0)
                  stg = opool.tile([128, total_m * D], f32, tag="stg")
                  for k in range(k0, k_end):
                    ilo, ihi = 2 * k, 2 * k + 1
                    sbase = (_off(ilo) - _off(2 * k0)) * D
                    np_lo = (F - 1) - ilo
                    np_hi = (F - 1) - ihi if ihi < F - 1 else 0
                    total = np_lo + np_hi

                    glo = [(s, min(8, np_lo - s)) for s in range(0, np_lo, 8)]
                    ghi = [(s, min(8, np_hi - s)) for s in range(0, np_hi, 8)]

                    if big_dve:
                        # One PSUM tile (up to ps_banks banks) per half-round;
                        # each group MM targets a bank-aligned slice; one DVE
                        # multiply per psum tile (chunks of ps_banks*8 pairs).
                        halves = [("lo", ilo, sbase, 0, np_lo, glo)]
                        if np_hi:
                            halves.append(
                                ("hi", ihi, sbase + np_lo * D, 64, np_hi, ghi)
                            )
                        chunk_pairs = ps_banks * 8
                        ps_tiles = {}  # (half, chunk_idx) -> tile
                        dve_jobs = []
                        for half, i, base, r0, npair, groups in halves:
                            for c0p in range(0, npair, chunk_pairs):
                                cp = min(chunk_pairs, npair - c0p)
                                pst = pspool.tile(
                                    [128, ps_banks * 512], f32, tag="ps", name="psbig"
                                )
                                ps_tiles[(half, c0p // chunk_pairs)] = pst
                                dve_jobs.append((half, i, base, c0p, cp, pst))
                        # interleave lo/hi MMs for PE row-half overlap
                        seq = []
                        for idx in range(max(len(glo), len(ghi))):
                            for half_info in halves:
                                if idx < len(half_info[5]):
                                    seq.append((half_info, half_info[5][idx]))
                        for (half, i, base, r0, npair, groups), (s, gs) in seq:
                            n = gs * D
                            gidx = (_CUM_EVEN[i] if half == "lo" else _CUM_ODD[i]) + s
                            fi = i // 2
                            lhsT = xt_buf[
                                r0 : r0 + 64,
                                fi * BS + t * 128 : fi * BS + t * 128 + 128,
                            ]
                            rhs = w_buf[r0 : r0 + 64, gidx * D : gidx * D + n]
                            pst = ps_tiles[(half, s // chunk_pairs)]
                            so = (s % chunk_pairs) * D
                            nc.tensor.matmul(
                                pst[:, so : so + n],
                                lhsT,
                                rhs,
                                start=True,
                                stop=True,
                            )
                        for half, i, base, c0p, cp, pst in dve_jobs:
                            nc.vector.tensor_mul(
                                out=stg[:, base + c0p * D : base + (c0p + cp) * D],
                                in0=pst[:, : cp * D],
                                in1=x_tile[
                                    :, (i + 1 + c0p) * D : (i + 1 + c0p + cp) * D
                                ].bitcast(f32),
                            )
                    else:
                        seq = []
                        for idx in range(max(len(glo), len(ghi))):
                            if idx < len(glo):
                                seq.append(("lo", glo[idx]))
                            if idx < len(ghi):
                                seq.append(("hi", ghi[idx]))

                        for half, (s, gs) in seq:
                            n = gs * D
                            if half == "lo":
                                i, base, r0 = ilo, sbase, 0
                                gidx = _CUM_EVEN[i] + s
                            else:
                                i, base, r0 = ihi, sbase + np_lo * D, 64
                                gidx = _CUM_ODD[i] + s
                            fi = i // 2
                            j0 = i + 1 + s
                            ps = pspool.tile([128, 512], f32, tag="ps")
                            lhsT = xt_buf[
                                r0 : r0 + 64,
                                fi * BS + t * 128 : fi * BS + t * 128 + 128,
                            ]
                            rhs = w_buf[r0 : r0 + 64, gidx * D : gidx * D + n]
                            nc.tensor.matmul(
                                ps[:, :n], lhsT, rhs, start=True, stop=True
                            )
                            nc.vector.tensor_mul(
                                out=stg[:, base + s * D : base + s * D + n],
                                in0=ps[:, :n],
                                in1=x_tile[:, j0 * D : j0 * D + n].bitcast(f32),
                            )

                    if k == k_end - 1:
                        c0 = _off(2 * k0) * D
                        nc.sync.dma_start(
                            y_d[t * 128 : (t + 1) * 128, c0 : c0 + total_m * D],
                            stg[:, :],
                        )

    nc.finalize()
    _NC_CACHE[key] = nc
    return nc


# ---------------------------------------------------------------------------
# fp16_v8 geometry: i-blocks, sections, psum tiles, matmul groups.
# ---------------------------------------------------------------------------
_V8_SPLIT_I = 10  # stg section A = blocks 0..9, B = 10..30
_V8_DIRECT_I = 20  # blocks >= this are DVE-direct from PSUM (no scalar evict)
_V8_PTILE = 2048  # psum tile cols for scalar-evicted ranges (4 banks)


def _v8_schedule():
    """Flat per-b_tile psum-tile schedule.

    Returns (sections, tiles):
      sections: {sec_i0: (sec_base_cols, sec_cols)} for the 2 stg tiles
      tiles: emission-ordered list of
        (sec_i0, sec_rel_col, pcols, groups, completed, direct_i) where
        groups = [(i, s, g, prel)] matmul groups (pair offset s, g pairs,
        psum-tile-relative col offset prel); completed = blocks whose last
        col lands in this tile (in-place DVE mul emitted after the evict);
        direct_i = block id for whole-block DVE-direct tiles (no evict).
    D-tiles are interleaved between S-tiles so the scalar engine (evicting
    S-tiles) and DVE (consuming D-tiles) always have independent live psum
    tiles with only bufs=2 (2 x 4 banks = all of PSUM).
    """
    sections = {}
    s_tiles, d_tiles = [], []
    for sec_i0, sec_i1 in ((0, _V8_SPLIT_I), (_V8_SPLIT_I, F - 1)):
        sec_cols = (_off(sec_i1) - _off(sec_i0)) * D
        sections[sec_i0] = (_off(sec_i0) * D, sec_cols)
        s_blocks = [i for i in range(sec_i0, sec_i1) if i < _V8_DIRECT_I]
        d_blocks = [i for i in range(sec_i0, sec_i1) if i >= _V8_DIRECT_I]
        stream = []  # (i, s) per pair in stream order
        for i in s_blocks:
            for s in range(F - 1 - i):
                stream.append((i, s))
        c = 0
        sec_rel = 0
        while c < len(stream):
            n = min(_V8_PTILE // D, len(stream) - c)  # pairs in this tile
            groups = []
            p = 0
            while p < n:
                i, s = stream[c + p]
                # group: same block, <=8 pairs, not crossing a psum bank
                g = 1
                while (
                    p + g < n
                    and stream[c + p + g][0] == i
                    and ((p + g) * D) % 512 != 0
                ):
                    g += 1
                groups.append((i, s, g, p * D))
                p += g
            tile_pairs = {stream[c + k] for k in range(n)}
            completed = tuple(
                i for i in s_blocks if (i, F - 2 - i) in tile_pairs
            )
            s_tiles.append((sec_i0, sec_rel, n * D, groups, completed, None))
            c += n
            sec_rel += n * D
        for i in d_blocks:
            np_i = F - 1 - i
            if np_i <= 0:
                continue
            groups = []
            p = 0
            while p < np_i:
                g = min(8, np_i - p, (512 - (p * D) % 512) // D)
                groups.append((i, p, g, p * D))
                p += g
            d_tiles.append(
                (sec_i0, (_off(i) - _off(sec_i0)) * D, np_i * D, groups, (), i)
            )
    # interleave: 2 S-tiles up front, then alternate S/D, S-tail last
    tiles = s_tiles[:2]
    si, di = 2, 0
    while si < len(s_tiles) or di < len(d_tiles):
        if si < len(s_tiles):
            tiles.append(s_tiles[si])
            si += 1
        if di < len(d_tiles):
            tiles.append(d_tiles[di])
            di += 1
    return sections, tiles


_V8_LIMIT_TILES = None  # debug: emit only the first N psum tiles per b_tile
_V8_LIMIT_T = None  # debug: emit only b_tiles < this
_V8_DBG_GROUPS = None  # debug: {tidx: ngroups} limit matmul groups of a tile
_V8_DBG_NOEVICT = ()  # debug: tile idxs whose scalar evict is skipped


# ---------------------------------------------------------------------------
# fp16_v9: single-half variant. HW finding (deterministic repro): a matmul
# with operands on partitions 0:64 immediately followed by one on partitions
# 64:128 writing the SAME psum bank crashes the device (the two PE row-halves
# run concurrently and collide on the bank write port; same-half pairs are
# fine because they serialize on the PE). v9 therefore packs xt and W for ALL
# features/pairs on partitions 0:64 (no even/odd split): every matmul is
# row-half 0, psum packing stays dense in natural triu pair order, and stg ==
# psum == output column order. stg is split into 4 sections per b_tile (one
# output DMA each, bufs=1) so SBUF fits: w 62KB + xt 16KB + x 8KB + stg 62KB.
# ---------------------------------------------------------------------------
_V9_SECTIONS = (0, 5, 10, 16, F - 1)  # block ranges of the 4 stg sections
_V9_DIRECT_I = 20  # blocks >= this are DVE-direct from PSUM (no scalar evict)
_V9_PTILE = 2048


def _v9_schedule():
    """Returns (sections, tiles).

    sections: list of (sec_idx, i0, i1, sec_base_cols, sec_cols)
    tiles: emission-ordered (sec_idx, sec_rel, pcols, groups, completed,
           direct_i) with groups = [(pair0, g, prel)] (g<=8 pairs, bank
           aligned); pair0 is the global pair index (w cols pair0*D..).
    """
    sections = []
    for si in range(len(_V9_SECTIONS) - 1):
        i0, i1 = _V9_SECTIONS[si], _V9_SECTIONS[si + 1]
        sections.append(
            (si, i0, i1, _off(i0) * D, (_off(i1) - _off(i0)) * D)
        )
    s_tiles, d_tiles = [], []
    for si, i0, i1, sec_base, sec_cols in sections:
        stream = []  # (i, global_pair) in stream order, scalar blocks only
        for i in range(i0, min(i1, _V9_DIRECT_I)):
            for s in range(F - 1 - i):
                stream.append((i, _off(i) + s))
        c = 0
        sec_rel = 0
        while c < len(stream):
            n = min(_V9_PTILE // D, len(stream) - c)
            groups = []
            p = 0
            while p < n:
                i, pair0 = stream[c + p]
                g = 1
                while (
                    p + g < n
                    and stream[c + p + g][0] == i
                    and ((p + g) * D) % 512 != 0
                ):
                    g += 1
                groups.append((pair0, g, p * D))
                p += g
            tile_blocks = {stream[c + k][0] for k in range(n)}
            last_i = stream[c + n - 1][0]
            completed = tuple(
                i for i in sorted(tile_blocks)
                if i < last_i or c + n == len(stream)
                or stream[c + n][0] != i
            )
            s_tiles.append((si, sec_rel, n * D, groups, completed, None))
            c += n
            sec_rel += n * D
        for i in range(max(i0, _V9_DIRECT_I), i1):
            np_i = F - 1 - i
            if np_i <= 0:
                continue
            groups = []
            p = 0
            while p < np_i:
                g = min(8, np_i - p, (512 - (p * D) % 512) // D)
                groups.append((_off(i) + p, g, p * D))
                p += g
            d_tiles.append(
                (si, (_off(i) - _off(i0)) * D, np_i * D, groups, (), i)
            )
    tiles = s_tiles[:2]
    si_, di = 2, 0
    while si_ < len(s_tiles) or di < len(d_tiles):
        if si_ < len(s_tiles):
            tiles.append(s_tiles[si_])
            si_ += 1
        if di < len(d_tiles):
            tiles.append(d_tiles[di])
            di += 1
    return sections, tiles


def _build_nc_v9(dtype_name="fp16_v9", repeat=1):
    import concourse.mybir as mybir
    import concourse.tile as tile
    from concourse import bacc

    key = (dtype_name, repeat)
    if key in _NC_CACHE:
        return _NC_CACHE[key]

    f32 = mybir.dt.float32
    f16 = mybir.dt.float16

    nc = bacc.Bacc("TRN2", target_bir_lowering=False, debug=False)
    x_d = nc.dram_tensor("x", [BS, F * D], f16, kind="ExternalInput")
    xt_d = nc.dram_tensor("xt", [64, F * BS], f16, kind="ExternalInput")
    w_d = nc.dram_tensor("w", [64, NPAIR * D], f16, kind="ExternalInput")
    y_d = nc.dram_tensor("y", [BS, PD], f16, kind="ExternalOutput")

    sections, ptiles = _v9_schedule()

    with tile.TileContext(nc) as tc:
        import contextlib

        with (
            tc.tile_pool(name="const", bufs=1) as const,
            tc.tile_pool(name="xp", bufs=2) as xpool,
            tc.tile_pool(name="ps", bufs=2, space="PSUM") as pspool,
            tc.tile_pool(name="stg", bufs=1) as spool,
            (tc.For_i(0, repeat, 1) if repeat > 1 else contextlib.nullcontext()),
        ):
            w_buf = const.tile([64, NPAIR * D], f16, tag="w")
            xt_buf = const.tile([64, F * BS], f16, tag="xt")
            x_tiles = {}
            for t in range(BS // 128):
                x_tiles[t] = xpool.tile([128, F * D], f16, tag="x", name=f"x{t}")

            # input DMAs in first-consumption order
            xtc = F * BS // 4  # 2048 cols per xt chunk (8 features)
            wc = NPAIR * D // 8  # 3968 cols per w chunk (62 pairs)
            nc.sync.dma_start(xt_buf[:, 0:xtc], xt_d[:, 0:xtc])
            nc.sync.dma_start(w_buf[:, 0:wc], w_d[:, 0:wc])
            nc.sync.dma_start(x_tiles[0][:, :], x_d[0:128, :])
            nc.sync.dma_start(w_buf[:, wc : 2 * wc], w_d[:, wc : 2 * wc])
            nc.sync.dma_start(xt_buf[:, xtc : 2 * xtc], xt_d[:, xtc : 2 * xtc])
            nc.sync.dma_start(w_buf[:, 2 * wc : 3 * wc], w_d[:, 2 * wc : 3 * wc])
            nc.sync.dma_start(x_tiles[1][:, :], x_d[128:256, :])
            nc.sync.dma_start(w_buf[:, 3 * wc : 4 * wc], w_d[:, 3 * wc : 4 * wc])
            nc.sync.dma_start(xt_buf[:, 2 * xtc : 3 * xtc], xt_d[:, 2 * xtc : 3 * xtc])
            nc.sync.dma_start(w_buf[:, 4 * wc : 5 * wc], w_d[:, 4 * wc : 5 * wc])
            nc.sync.dma_start(xt_buf[:, 3 * xtc :], xt_d[:, 3 * xtc :])
            for q in range(5, 8):
                nc.sync.dma_start(
                    w_buf[:, q * wc : (q + 1) * wc], w_d[:, q * wc : (q + 1) * wc]
                )

            for t in range(BS // 128):
                x_tile = x_tiles[t]
                stgs = {
                    si: spool.tile(
                        [128, sec_cols], f16, tag=f"stg{si}", name=f"stg{si}_{t}"
                    )
                    for si, i0, i1, sec_base, sec_cols in sections
                }
                remaining = {
                    si: sections[si][4] for si, *_ in sections
                }
                for si, sec_rel, pcols, groups, completed, direct_i in ptiles:
                    stg = stgs[si]
                    _, i0, i1, sec_base, sec_cols = sections[si]
                    pst = pspool.tile([128, _V9_PTILE], f32, tag="ps", name="ps")
                    for pair0, g, prel in groups:
                        n = g * D
                        i_blk = _pair_i()[pair0]
                        lhsT = xt_buf[
                            :, i_blk * BS + t * 128 : i_blk * BS + t * 128 + 128
                        ]
                        rhs = w_buf[:, pair0 * D : pair0 * D + n]
                        nc.tensor.matmul(
                            pst[:, prel : prel + n], lhsT, rhs,
                            start=True, stop=True,
                        )
                    if direct_i is None:
                        nc.scalar.copy(
                            stg[:, sec_rel : sec_rel + pcols], pst[:, :pcols]
                        )
                        for i in completed:
                            b0 = _off(i) * D - sec_base
                            np_i = F - 1 - i
                            nc.vector.tensor_mul(
                                out=stg[:, b0 : b0 + np_i * D],
                                in0=stg[:, b0 : b0 + np_i * D],
                                in1=x_tile[:, (i + 1) * D : (i + 1 + np_i) * D],
                            )
                    else:
                        i = direct_i
                        np_i = F - 1 - i
                        nc.vector.tensor_mul(
                            out=stg[:, sec_rel : sec_rel + np_i * D],
                            in0=pst[:, : np_i * D],
                            in1=x_tile[:, (i + 1) * D : (i + 1 + np_i) * D],
                        )
                    remaining[si] -= pcols
                    if remaining[si] == 0:
                        nc.sync.dma_start(
                            y_d[
                                t * 128 : (t + 1) * 128,
                                sec_base : sec_base + sec_cols,
                            ],
                            stg[:, :],
                        )

    nc.finalize()
    _NC_CACHE[key] = nc
    return nc


def _build_nc_v8(dtype_name="fp16_v8", repeat=1):
    import concourse.mybir as mybir
    import concourse.tile as tile
    from concourse import bacc

    key = (dtype_name, repeat)
    if key in _NC_CACHE:
        return _NC_CACHE[key]

    f32 = mybir.dt.float32
    f16 = mybir.dt.float16

    nc = bacc.Bacc("TRN2", target_bir_lowering=False, debug=False)
    x_d = nc.dram_tensor("x", [BS, F * D], f16, kind="ExternalInput")
    xt_d = nc.dram_tensor("xt", [128, 16 * BS], f16, kind="ExternalInput")
    w_d = nc.dram_tensor("w", [128, _N_EVEN * D], f16, kind="ExternalInput")
    y_d = nc.dram_tensor("y", [BS, PD], f16, kind="ExternalOutput")

    sections, ptiles = _v8_schedule()

    with tile.TileContext(nc) as tc:
        import contextlib

        with (
            tc.tile_pool(name="const", bufs=1) as const,
            tc.tile_pool(name="xp", bufs=2) as xpool,
            tc.tile_pool(name="ps", bufs=2, space="PSUM") as pspool,
            tc.tile_pool(name="stg", bufs=2) as spool,
            (tc.For_i(0, repeat, 1) if repeat > 1 else contextlib.nullcontext()),
        ):
            w_buf = const.tile([128, _N_EVEN * D], f16, tag="w")
            xt_buf = const.tile([128, 16 * BS], f16, tag="xt")
            x_tiles = {}
            for t in range(BS // 128):
                x_tiles[t] = xpool.tile([128, F * D], f16, tag="x", name=f"x{t}")

            # input DMAs in first-consumption order
            xtc = 16 * BS // 4  # 1024 cols per xt chunk
            wc = _N_EVEN * D // 8  # 2048 cols per w chunk
            nc.sync.dma_start(xt_buf[:, 0:xtc], xt_d[:, 0:xtc])
            nc.sync.dma_start(w_buf[:, 0:wc], w_d[:, 0:wc])
            nc.sync.dma_start(x_tiles[0][:, :], x_d[0:128, :])
            nc.sync.dma_start(w_buf[:, wc : 2 * wc], w_d[:, wc : 2 * wc])
            nc.sync.dma_start(xt_buf[:, xtc : 2 * xtc], xt_d[:, xtc : 2 * xtc])
            nc.sync.dma_start(w_buf[:, 2 * wc : 3 * wc], w_d[:, 2 * wc : 3 * wc])
            nc.sync.dma_start(x_tiles[1][:, :], x_d[128:256, :])
            nc.sync.dma_start(w_buf[:, 3 * wc : 4 * wc], w_d[:, 3 * wc : 4 * wc])
            nc.sync.dma_start(xt_buf[:, 2 * xtc : 3 * xtc], xt_d[:, 2 * xtc : 3 * xtc])
            nc.sync.dma_start(w_buf[:, 4 * wc : 5 * wc], w_d[:, 4 * wc : 5 * wc])
            nc.sync.dma_start(xt_buf[:, 3 * xtc :], xt_d[:, 3 * xtc :])
            for q in range(5, 7):
                nc.sync.dma_start(
                    w_buf[:, q * wc : (q + 1) * wc], w_d[:, q * wc : (q + 1) * wc]
                )
            # last chunk: odd half (rows 64:128) is zero-padded past _N_ODD*D
            nc.sync.dma_start(w_buf[0:64, 7 * wc :], w_d[0:64, 7 * wc :])
            nc.sync.dma_start(
                w_buf[64:128, 7 * wc : _N_ODD * D], w_d[64:128, 7 * wc : _N_ODD * D]
            )

            for t in range(BS // 128):
                if _V8_LIMIT_T is not None and t >= _V8_LIMIT_T:
                    break
                x_tile = x_tiles[t]
                stgs = {
                    sec_i0: spool.tile(
                        [128, sec_cols], f16, tag=f"stg{sec_i0}",
                        name=f"stg{sec_i0}_{t}",
                    )
                    for sec_i0, (_, sec_cols) in sections.items()
                }
                remaining = {
                    sec_i0: sum(
                        pt[2] for pt in ptiles if pt[0] == sec_i0
                    )
                    for sec_i0 in sections
                }
                for tidx, (sec_i0, sec_rel, pcols, groups, completed, direct_i) in (
                    enumerate(ptiles)
                ):
                    if _V8_LIMIT_TILES is not None and tidx >= _V8_LIMIT_TILES:
                        break
                    stg = stgs[sec_i0]
                    pst = pspool.tile([128, _V8_PTILE], f32, tag="ps", name="ps")
                    if _V8_DBG_GROUPS and tidx in _V8_DBG_GROUPS:
                        groups = groups[: _V8_DBG_GROUPS[tidx]]
                    if tidx in _V8_DBG_NOEVICT:
                        for i, s, g, prel in groups:
                            n = g * D
                            r0 = 0 if i % 2 == 0 else 64
                            gidx = (
                                _CUM_EVEN[i] if i % 2 == 0 else _CUM_ODD[i]
                            ) + s
                            fi = i // 2
                            nc.tensor.matmul(
                                pst[:, prel : prel + n],
                                xt_buf[
                                    r0 : r0 + 64,
                                    fi * BS + t * 128 : fi * BS + t * 128 + 128,
                                ],
                                w_buf[r0 : r0 + 64, gidx * D : gidx * D + n],
                                start=True,
                                stop=True,
                            )
                        remaining[sec_i0] -= pcols
                        continue
                    for i, s, g, prel in groups:
                        n = g * D
                        if i % 2 == 0:
                            r0, gidx = 0, _CUM_EVEN[i] + s
                        else:
                            r0, gidx = 64, _CUM_ODD[i] + s
                        fi = i // 2
                        lhsT = xt_buf[
                            r0 : r0 + 64,
                            fi * BS + t * 128 : fi * BS + t * 128 + 128,
                        ]
                        rhs = w_buf[r0 : r0 + 64, gidx * D : gidx * D + n]
                        nc.tensor.matmul(
                            pst[:, prel : prel + n], lhsT, rhs,
                            start=True, stop=True,
                        )
                    sec_base = sections[sec_i0][0]
                    if "dbg1" in dtype_name:
                        # debug: scalar-evict everything, no DVE muls
                        nc.scalar.copy(
                            stg[:, sec_rel : sec_rel + pcols], pst[:, :pcols]
                        )
                    elif direct_i is None:
                        nc.scalar.copy(
                            stg[:, sec_rel : sec_rel + pcols], pst[:, :pcols]
                        )
                        for i in completed:
                            b0 = _off(i) * D - sec_base
                            np_i = F - 1 - i
                            nc.vector.tensor_mul(
                                out=stg[:, b0 : b0 + np_i * D],
                                in0=stg[:, b0 : b0 + np_i * D],
                                in1=x_tile[:, (i + 1) * D : (i + 1 + np_i) * D],
                            )
                    else:
                        i = direct_i
                        np_i = F - 1 - i
                        nc.vector.tensor_mul(
                            out=stg[:, sec_rel : sec_rel + np_i * D],
                            in0=pst[:, : np_i * D],
                            in1=x_tile[:, (i + 1) * D : (i + 1 + np_i) * D],
                        )
                    remaining[sec_i0] -= pcols
                    if remaining[sec_i0] == 0:
                        sec_cols = sections[sec_i0][1]
                        nc.sync.dma_start(
                            y_d[
                                t * 128 : (t + 1) * 128,
                                sec_base : sec_base + sec_cols,
                            ],
                            stg[:, :],
                        )

    nc.finalize()
    _NC_CACHE[key] = nc
    return nc


def _prep_inputs(inputs, W, host_xt=True, dtype_name=None):
    dn = dtype_name or DTYPE
    st_dt = np.float16 if dn.startswith("fp16") else np.float32
    inputs = np.ascontiguousarray(np.asarray(inputs, dtype=np.float32))
    W = np.ascontiguousarray(np.asarray(W, dtype=np.float32))

    if "v9" in dn:
        # single-half packing: xt [64, F*BS] (col = f*BS + b), w [64, P*D]
        w_packed = np.ascontiguousarray(
            W.transpose(1, 0, 2).reshape(64, NPAIR * D).astype(st_dt)
        )
        in_maps = []
        for c in range(NCORES):
            xs = inputs[c * BS : (c + 1) * BS].astype(st_dt)  # [256, 32, 64]
            x_flat = np.ascontiguousarray(xs.reshape(BS, F * D))
            xt = np.ascontiguousarray(
                xs.transpose(2, 1, 0).reshape(64, F * BS)
            )
            in_maps.append({"x": x_flat, "w": w_packed, "xt": xt})
        return in_maps

    even_p = [p for p, i in enumerate(_pair_i()) if i % 2 == 0]
    odd_p = [p for p, i in enumerate(_pair_i()) if i % 2 == 1]
    w_packed = np.zeros((128, _N_EVEN * D), dtype=st_dt)
    w_packed[0:64, :] = W[even_p].transpose(1, 0, 2).reshape(64, _N_EVEN * D)
    w_packed[64:128, : _N_ODD * D] = (
        W[odd_p].transpose(1, 0, 2).reshape(64, _N_ODD * D)
    )

    in_maps = []
    for c in range(NCORES):
        xs = inputs[c * BS : (c + 1) * BS].astype(st_dt)  # [256, 32, 64]
        x_flat = np.ascontiguousarray(xs.reshape(BS, F * D))
        m = {"x": x_flat, "w": w_packed}
        if not host_xt:
            m["ident"] = np.eye(128, dtype=np.float32)
        if host_xt:
            xtt = xs.transpose(2, 1, 0)  # [64, 32, 256]
            xt = np.empty((128, 16 * BS), dtype=st_dt)
            xt[0:64, :] = np.ascontiguousarray(xtt[:, 0::2, :]).reshape(64, 16 * BS)
            xt[64:128, :] = np.ascontiguousarray(xtt[:, 1::2, :]).reshape(64, 16 * BS)
            m["xt"] = xt
        in_maps.append(m)
    return in_maps


_PAIR_I = None


def _pair_i():
    global _PAIR_I
    if _PAIR_I is None:
        _PAIR_I = [i for i in range(F) for _ in range(i + 1, F)]
    return _PAIR_I


def _run(inputs, W, trace=False, trace_cores=None, dtype_name=None):
    from concourse.bass_utils import run_bass_kernel_spmd

    dn = dtype_name or DTYPE
    nc = _build_nc(dn)
    in_maps = _prep_inputs(inputs, W, host_xt="_notr" not in dn, dtype_name=dn)
    res = run_bass_kernel_spmd(
        nc,
        in_maps,
        core_ids=list(range(NCORES)),
        trace=trace,
        trace_cores=trace_cores,
    )
    out = np.concatenate([res.results[c]["y"] for c in range(NCORES)], axis=0)
    if out.dtype != np.float32:
        out = out.astype(np.float32)
    return out, res


def kernel(inputs, W):
    out, _ = _run(inputs, W, trace=False)
    return out

